# revision 46
# baseline (speedup 1.0000x reference)
"""3-layer GATv2 (PyG GATv2Conv semantics) on 8 Trainium2 NeuronCores.

Distribution: nodes sharded 12500/core; edges (incl. self-loops) partitioned
by dst core, grouped into 128-dst-node blocks. Per layer:
  phase A: [xl|xr] = h @ [Wl|Wr] for local nodes (PE, lhsT = feature-major
           h_T), rows stored bf16, AllGather -> every core holds all rows.
  phase B: per superblock of blocks, batched dma_gather (int16 idx; global
           src rows via even/odd row split to fit int16, local dst rows),
           z = xl[src]+xr[dst], lrelu (DVE max(0.2z,z)), score = reduce(z*a),
           w = exp(score) (softmax w/o max-subtraction; scores are O(1)),
           per-block indicator matmul S.T @ [w*xg | w] accumulates weighted
           sums + denominators in PSUM. Divide, bias, ELU, and emit h_T
           feature-major for the next layer. Layer 3: divide then head-mean,
           f32 shard output, host concat.
"""
import sys
sys.path.insert(0, "/opt/trn_rl_repo")
import numpy as np
import ml_dtypes

N = 100000
E = 800000
NCORES = 8
SHARD = N // NCORES        # 12500
P = 128
NBLK = (SHARD + P - 1) // P  # 98
SB = 6                      # node blocks per superblock
FIN = 64
H = 4
C1, C3 = 16, 32
F1 = H * C1                # 64
F3 = H * C3                # 128
NEG_SLOPE = 0.2

BF16 = ml_dtypes.bfloat16

_cache = {}


class Meta:
    pass


def _preprocess(edge_index):
    """Sort edges by dst; per (core, block) split by src parity; pad each run
    to x128 (uniform across cores). Group order per superblock:
    [even-groups (block-major) | odd-groups (block-major)].
    Returns per-core idx arrays + graph meta (uniform across cores)."""
    src = np.concatenate([edge_index[0], np.arange(N, dtype=np.int32)])
    dst = np.concatenate([edge_index[1], np.arange(N, dtype=np.int32)])
    order = np.argsort(dst, kind="stable")
    src_s = src[order].astype(np.int64)
    dst_s = dst[order].astype(np.int64)
    # remap src node ids to the half-split AllGather table layout: the two
    # half-shard collectives concat rank-major, so table row of node (c, p) is
    # c*HALF + p for p < HALF, else N/2 + c*HALF + (p - HALF).
    HALF = SHARD // 2
    c_ = src_s // SHARD
    p_ = src_s % SHARD
    src_s = np.where(p_ < HALF, c_ * HALF + p_,
                     N // 2 + c_ * HALF + (p_ - HALF))

    core = dst_s // SHARD
    blk = (dst_s - core * SHARD) // P
    key = core * NBLK + blk
    cnt = np.bincount(key, minlength=NCORES * NBLK).reshape(NCORES, NBLK)
    starts = np.concatenate([[0], np.cumsum(cnt.reshape(-1))])

    # per (core, block, class): src%4 class run lengths (int16 idx = src//4)
    NCLS = 4
    gc = np.zeros((NCORES, NBLK, NCLS), np.int64)
    runs = {}
    for c in range(NCORES):
        for b in range(NBLK):
            i = c * NBLK + b
            s, e = starts[i], starts[i + 1]
            sr, dr = src_s[s:e], dst_s[s:e]
            cls = sr % NCLS
            for r in range(NCLS):
                sel = cls == r
                runs[(c, b, r)] = (sr[sel], dr[sel])
                gc[c, b, r] = sel.sum()
    Gc = np.maximum(1, -(-gc.max(axis=0) // P))   # [NBLK, NCLS] groups per run

    m = Meta()
    m.NCLS = NCLS
    m.NSB = (NBLK + SB - 1) // SB
    m.sb_blocks = [list(range(s * SB, min(NBLK, (s + 1) * SB))) for s in range(m.NSB)]
    # pooled slots: per (sb, class) one contiguous run over all the sb's
    # blocks (block-sorted), padded to x128 only at the class-run end.
    # Slots may straddle blocks; each (slot, block) pair gets its own masked
    # indicator region in sw/stw so scatter/gather matmuls stay block-pure.
    m.sb_cls_off = []   # per sb: [o0..o4] class slot offsets
    m.sb_g = []         # per sb: slot count
    m.sb_pairs = []     # per sb: pair count
    m.sb_slot_pairs = []  # per sb: slot -> [(pair_idx, block)]
    m.blk_pairs = [[] for _ in range(NBLK)]  # block -> [(sb, pair_idx, slot)]
    for s, bs in enumerate(m.sb_blocks):
        offs = [0]
        slot_blocks = {}   # slot -> set of blocks (union over cores)
        slot = 0
        for r in range(NCLS):
            csum = gc[:, bs, r]                      # [NCORES, len(bs)]
            tot = int(csum.sum(axis=1).max())
            G = max(1, -(-tot // P))
            for c in range(NCORES):
                pos = 0
                for i, b in enumerate(bs):
                    n = int(csum[c, i])
                    if n:
                        for sl in range(pos // P, (pos + n - 1) // P + 1):
                            slot_blocks.setdefault(slot + sl, set()).add(b)
                    pos += n
            offs.append(slot + G)
            slot = slot + G
        m.sb_cls_off.append(offs)
        m.sb_g.append(slot)
        spairs = [[] for _ in range(slot)]
        pcnt = 0
        for sl in range(slot):
            for b in sorted(slot_blocks.get(sl, ())):
                spairs[sl].append((pcnt, b))
                m.blk_pairs[b].append((s, pcnt, sl))
                pcnt += 1
        m.sb_slot_pairs.append(spairs)
        m.sb_pairs.append(pcnt)
    m.SBGmax = max(m.sb_g)
    m.SBPmax = max(m.sb_pairs)
    m.Gtot = sum(m.sb_g)
    m.Ptot = sum(m.sb_pairs)
    sb_goff = np.concatenate([[0], np.cumsum(m.sb_g)])
    m.sb_goff = [int(v) for v in sb_goff]
    pb_goff = np.concatenate([[0], np.cumsum(m.sb_pairs)])
    m.sb_poff = [int(v) for v in pb_goff]

    # per-core fills: flat lanes in (sb, slot, lane) order; indicator arrays
    # in (sb, pair, lane-in-slot) order.
    src_idx = np.zeros((NCORES, m.Gtot * P), np.int16)   # value: src//4
    stw = np.zeros((NCORES, P, m.Ptot * P), BF16)        # [n, pair*P+lane]
    sw = np.zeros((NCORES, P, m.Ptot * P), BF16)         # [lane, pair*P+n]
    for c in range(NCORES):
        for s, bs in enumerate(m.sb_blocks):
            base = m.sb_goff[s] * P
            pbase = m.sb_poff[s]
            pair_of = {(sl, b): p for sl, prs in enumerate(m.sb_slot_pairs[s])
                       for p, b in prs}
            for r in range(NCLS):
                pos = m.sb_cls_off[s][r] * P
                for b in bs:
                    sr, dr = runs[(c, b, r)]
                    n = len(sr)
                    if n == 0:
                        continue
                    lanes = base + pos + np.arange(n)
                    src_idx[c, lanes] = sr // NCLS
                    relv = (dr - c * SHARD - b * P).astype(np.int64)
                    sl_arr = (pos + np.arange(n)) // P
                    lin = (pos + np.arange(n)) % P
                    pidx = np.array([pair_of[(int(sl), b)] for sl in sl_arr],
                                    dtype=np.int64) + pbase
                    stw[c, relv, pidx * P + lin] = 1
                    sw[c, lin, pidx * P + relv] = 1
                    pos += n
    m.stw = stw
    m.sw = sw

    def wrap(a):
        # a: [NCORES, Gtot*P] -> [NCORES, 128, Gtot*8]
        w = a.reshape(NCORES, m.Gtot * P // 16, 16).transpose(0, 2, 1)
        return np.tile(w, (1, 8, 1)).copy()

    m.src_w = wrap(src_idx)
    return m


def _build_program(m):
    import os
    import concourse.bass as bass
    import concourse.bacc as bacc
    import concourse.tile as tile
    from concourse import mybir, library_config

    STAGE = int(os.environ.get("BK_STAGE", "99"))
    NLAYER = int(os.environ.get("BK_NLAYER", "3"))
    NTILE = int(os.environ.get("BK_NTILE", "9999"))

    bf16, f32, i16 = mybir.dt.bfloat16, mybir.dt.float32, mybir.dt.int16
    AF = mybir.ActivationFunctionType
    OP = mybir.AluOpType
    X = mybir.AxisListType.X

    nc = bacc.Bacc("TRN2", target_bir_lowering=False)

    WI = m.Gtot * P // 16
    xT_d = nc.dram_tensor("xT", [FIN, SHARD], bf16, kind="ExternalInput")
    srcw_d = nc.dram_tensor("srcw", [P, WI], i16, kind="ExternalInput")
    stw_d = nc.dram_tensor("stw", [P, m.Ptot * P], bf16, kind="ExternalInput")
    sw_d = nc.dram_tensor("sw", [P, m.Ptot * P], bf16, kind="ExternalInput")
    W_d = [nc.dram_tensor(f"W{l}", [FIN, 2 * (F1 if l < 3 else F3)], bf16, kind="ExternalInput") for l in (1, 2, 3)]
    arep_d = [nc.dram_tensor(f"arep{l}", [P, F1 if l < 3 else F3], bf16, kind="ExternalInput") for l in (1, 2, 3)]
    brep_d = [nc.dram_tensor("brep1", [P, F1], bf16, kind="ExternalInput"),
              nc.dram_tensor("brep2", [P, F1], bf16, kind="ExternalInput"),
              nc.dram_tensor("brep3", [P, C3], f32, kind="ExternalInput")]
    ident_d = nc.dram_tensor("ident", [P, P], bf16, kind="ExternalInput")
    out_d = nc.dram_tensor("out_shard", [SHARD, C3], f32, kind="ExternalOutput")

    def internal(name, shape, dt, shared=False):
        return nc.dram_tensor(name, shape, dt, kind="Internal",
                              addr_space="Shared" if shared else "Local")

    hT_next = [internal(f"hT{l}", [F1, SHARD], bf16) for l in (1, 2)]
    xlr_sh = [internal(f"xlrsh{l}", [SHARD, 2 * (F1 if l < 3 else F3)], bf16) for l in (1, 2, 3)]
    xlr_rows_cc = [internal(f"xlrrowscc{l}", [N, 2 * F1], bf16, shared=True) for l in (1, 2)]
    xl3_sh = internal("xl3sh", [SHARD, F3], bf16)
    xl3_cc = internal("xl3cc", [N, F3], bf16, shared=True)

    RG = [list(range(NCORES))]

    with tile.TileContext(nc) as tc:
        nc.gpsimd.load_library(library_config.mlp)
        with tc.tile_pool(name="const", bufs=1) as cpool, \
             tc.tile_pool(name="work", bufs=2) as wpool, \
             tc.tile_pool(name="mmA", bufs=2) as apool, \
             tc.tile_pool(name="tail", bufs=2) as tpool, \
             tc.tile_pool(name="hb", bufs=2 * SB + 2) as hpool, \
             tc.tile_pool(name="psA", bufs=2, space="PSUM") as ppA, \
             tc.tile_pool(name="psB", bufs=2, space="PSUM") as ppB, \
             tc.tile_pool(name="psXR", bufs=2, space="PSUM") as ppXR, \
             tc.tile_pool(name="psT", bufs=2, space="PSUM") as ppT:

            ident = cpool.tile([P, P], bf16)
            nc.sync.dma_start(ident[:], ident_d[:])
            W_sb, arep_sb, brep_sb = [], [], []
            for li in range(3):
                Fl = F1 if li < 2 else F3
                w = cpool.tile([FIN, 2 * Fl], bf16, tag=f"W{li}")
                nc.sync.dma_start(w[:], W_d[li][:])
                W_sb.append(w)
                a = cpool.tile([P, Fl], bf16, tag=f"arep{li}")
                nc.sync.dma_start(a[:], arep_d[li][:])
                arep_sb.append(a)
                b = cpool.tile([P, F1 if li < 2 else C3], bf16 if li < 2 else f32, tag=f"brep{li}")
                nc.sync.dma_start(b[:], brep_d[li][:])
                brep_sb.append(b)

            for li in range(3):
                l3 = (li == 2)
                Fl = F3 if l3 else F1
                Cl = C3 if l3 else C1
                FE = 2 * Fl            # row width of xlr tensors

                # AllGather halves: first half fires mid-phase-A and overlaps
                # the remaining feature-transform tiles; rank-major half
                # concat matches the src_s table-row remap in _preprocess.
                HALF = SHARD // 2

                def emit_ag(lo, hi, li=li, l3=l3):
                    if not l3:
                        nc.gpsimd.collective_compute(
                            "AllGather", mybir.AluOpType.bypass, replica_groups=RG,
                            ins=[xlr_sh[li][lo:hi, :]],
                            outs=[xlr_rows_cc[li][lo * NCORES:hi * NCORES, :]])
                    else:
                        nc.gpsimd.collective_compute(
                            "AllGather", mybir.AluOpType.bypass, replica_groups=RG,
                            ins=[xl3_sh[lo:hi, :]],
                            outs=[xl3_cc[lo * NCORES:hi * NCORES, :]])

                # ---- phase A ----
                for t in range(min(NBLK, NTILE)):
                    n0 = t * P
                    mm = min(P, SHARD - n0)
                    hTt = apool.tile([FIN, P], bf16, tag="hTt")
                    src_h = xT_d if li == 0 else hT_next[li - 1]
                    nc.sync.dma_start(hTt[:, :mm], src_h[:, n0:n0 + mm])
                    psA = ppA.tile([P, 2 * F3], f32, tag="psA", space="PSUM")
                    nc.tensor.matmul(psA[:mm, :FE], lhsT=hTt[:, :mm],
                                     rhs=W_sb[li][:], start=True, stop=True)
                    xlr = apool.tile([P, 2 * F3], bf16, tag="xlr")
                    nc.scalar.copy(xlr[:mm, :FE], psA[:mm, :FE])
                    nc.sync.dma_start(xlr_sh[li][n0:n0 + mm, :], xlr[:mm, :FE])
                    if l3:
                        nc.sync.dma_start(xl3_sh[n0:n0 + mm, :], xlr[:mm, :F3])
                    if n0 + mm >= HALF and n0 < HALF:
                        emit_ag(0, HALF)
                emit_ag(HALF, SHARD)

                if not l3:
                    src_tabs = [xlr_rows_cc[li][r::4, :] for r in range(m.NCLS)]
                    GELEM, GSTEP = FE, 4 * FE
                else:
                    src_tabs = [xl3_cc[r::4, :] for r in range(m.NCLS)]
                    GELEM, GSTEP = F3, 4 * F3

                # ---- phase B ----
                if STAGE < 3 or li >= NLAYER:
                    continue

                def emit_tail(b, hbB, li=li, l3=l3, Fl=Fl, Cl=Cl):
                    n0 = b * P
                    mm = min(P, SHARD - n0)
                    rec = tpool.tile([P, H], f32, tag="rec")
                    nc.vector.reciprocal(rec[:], hbB[:, Fl:Fl + H])
                    if not l3:
                        hb = tpool.tile([P, F1], bf16, tag="hb")
                        nc.vector.tensor_tensor(
                            out=hb[:].rearrange("p (h c) -> p h c", h=H),
                            in0=hbB[:, :Fl].rearrange("p (h c) -> p h c", h=H),
                            in1=rec[:, :, None].to_broadcast([P, H, Cl]), op=OP.mult)
                        nc.vector.tensor_tensor(out=hb[:], in0=hb[:],
                                                in1=brep_sb[li][:], op=OP.add)
                        rp = tpool.tile([P, F1], bf16, tag="rp")
                        nc.scalar.activation(rp[:], hb[:], AF.Relu)
                        xm = tpool.tile([P, F1], bf16, tag="xm")
                        nc.scalar.activation(xm[:], hb[:], AF.Relu, scale=-1.0)
                        ex = tpool.tile([P, F1], f32, tag="ex")
                        nc.scalar.activation(ex[:], xm[:], AF.Exp, scale=-1.0)
                        ho = tpool.tile([P, F1], bf16, tag="ho")
                        nc.vector.scalar_tensor_tensor(
                            out=ho[:], in0=ex[:], scalar=-1.0, in1=rp[:],
                            op0=OP.add, op1=OP.add)
                        psT = ppT.tile([F1, P], bf16, tag="psT", space="PSUM")
                        nc.tensor.transpose(psT[:, :mm], ho[:mm, :], ident[:mm, :mm])
                        hTn = tpool.tile([F1, P], bf16, tag="hTn")
                        nc.scalar.copy(hTn[:, :mm], psT[:, :mm])
                        nc.sync.dma_start(hT_next[li][:, n0:n0 + mm], hTn[:, :mm])
                    else:
                        o3 = tpool.tile([P, F3], f32, tag="o3")
                        nc.vector.tensor_tensor(
                            out=o3[:].rearrange("p (h c) -> p h c", h=H),
                            in0=hbB[:, :Fl].rearrange("p (h c) -> p h c", h=H),
                            in1=rec[:, :, None].to_broadcast([P, H, Cl]), op=OP.mult)
                        m01 = tpool.tile([P, C3], f32, tag="m01")
                        nc.vector.tensor_tensor(out=m01[:], in0=o3[:, 0:C3],
                                                in1=o3[:, C3:2 * C3], op=OP.add)
                        m23 = tpool.tile([P, C3], f32, tag="m23")
                        nc.vector.tensor_tensor(out=m23[:], in0=o3[:, 2 * C3:3 * C3],
                                                in1=o3[:, 3 * C3:4 * C3], op=OP.add)
                        ms = tpool.tile([P, C3], f32, tag="ms")
                        nc.vector.tensor_tensor(out=ms[:], in0=m01[:], in1=m23[:], op=OP.add)
                        of = tpool.tile([P, C3], f32, tag="of")
                        nc.vector.scalar_tensor_tensor(
                            out=of[:], in0=ms[:], scalar=0.25, in1=brep_sb[2][:],
                            op0=OP.mult, op1=OP.add)
                        nc.sync.dma_start(out_d[n0:n0 + mm, :], of[:mm, :])

                pending = []
                bsz = 4 if l3 else 8           # slots per PSUM xr batch
                for s in range(m.NSB):
                    SG = m.sb_g[s]
                    go = m.sb_goff[s]
                    wo = go * P // 16
                    bs0 = m.sb_blocks[s][0]
                    NP = m.sb_pairs[s]
                    po = m.sb_poff[s]
                    srcw_t = wpool.tile([P, m.SBGmax * 8], i16, tag="srcw")
                    nc.sync.dma_start(srcw_t[:, :SG * 8], srcw_d[:, wo:wo + SG * 8])
                    stt = wpool.tile([P, m.SBPmax * P], bf16, tag="stt")
                    nc.scalar.dma_start(stt[:, :NP * P], stw_d[:, po * P:(po + NP) * P])
                    S = wpool.tile([P, m.SBPmax, P], bf16, tag="S")
                    nc.scalar.dma_start(
                        S[:, :NP, :],
                        sw_d[:, po * P:(po + NP) * P].rearrange("p (g n) -> p g n", g=NP))
                    xrb = wpool.tile([P, SB, F3], bf16, tag="xrb")
                    for i, b in enumerate(m.sb_blocks[s]):
                        n0 = b * P
                        mm = min(P, SHARD - n0)
                        nc.sync.dma_start(xrb[:mm, i, :Fl],
                                          xlr_sh[li][n0:n0 + mm, Fl:2 * Fl])
                    xg = wpool.tile([P, m.SBGmax, F3 if l3 else FE], bf16, tag="xg")
                    offs = m.sb_cls_off[s]
                    for r in range(m.NCLS):
                        nr = (offs[r + 1] - offs[r]) * P
                        if nr == 0:
                            continue
                        nc.gpsimd.dma_gather(
                            out_ap=xg[:, offs[r]:offs[r + 1], :GELEM], in_ap=src_tabs[r],
                            idxs_ap=srcw_t[:, offs[r] * 8:offs[r + 1] * 8],
                            num_idxs=nr, num_idxs_reg=nr, elem_size=GELEM,
                            elem_step=GSTEP, single_packet=False)
                    if STAGE < 4:
                        continue
                    xgv = xg[:, :SG, :Fl]                      # xl[src]
                    # z = xl[src] + xr[dst]: xr[dst] per edge-lane comes from
                    # per-slot indicator matmuls (S_T.T @ xr_block) in PSUM;
                    # z lands in the unused xr half of xg (l1/2) or in the
                    # wlhs region that the weighted-mult later overwrites (l3).
                    wlhs = wpool.tile([P, m.SBGmax, F3 + H], bf16, tag="wlhs")
                    if not l3:
                        zv = xg[:, :SG, Fl:FE]
                    else:
                        zv = wlhs[:, :SG, :F3]
                    for c0 in range(0, SG, bsz):
                        nb = min(bsz, SG - c0)
                        psXR = ppXR.tile([P, bsz, F3 if l3 else F1], f32,
                                         tag="psXR", space="PSUM")
                        for j in range(nb):
                            sl = c0 + j
                            prs = m.sb_slot_pairs[s][sl]
                            for k, (p, b) in enumerate(prs):
                                nc.tensor.matmul(psXR[:, j, :Fl],
                                                 lhsT=stt[:, p * P:(p + 1) * P],
                                                 rhs=xrb[:, b - bs0, :Fl],
                                                 start=(k == 0),
                                                 stop=(k == len(prs) - 1))
                        nc.vector.tensor_tensor(
                            out=zv[:, c0:c0 + nb, :], in0=xgv[:, c0:c0 + nb, :],
                            in1=psXR[:, :nb, :Fl], op=OP.add)
                    nc.vector.scalar_tensor_tensor(
                        out=zv, in0=zv, scalar=NEG_SLOPE, in1=zv,
                        op0=OP.mult, op1=OP.max)
                    nc.vector.tensor_tensor(
                        out=zv, in0=zv,
                        in1=arep_sb[li][:, None, :].to_broadcast([P, SG, Fl]),
                        op=OP.mult)
                    score = wpool.tile([P, m.SBGmax * H], f32, tag="score")
                    nc.vector.tensor_reduce(
                        out=score[:, :SG * H],
                        in_=zv.rearrange("p g (h c) -> p g h c", h=H),
                        axis=X, op=OP.add)
                    nc.scalar.activation(
                        wlhs[:, :SG, Fl:Fl + H],
                        score[:, :SG * H].rearrange("p (g h) -> p g h", g=SG),
                        AF.Exp)
                    nc.vector.tensor_tensor(
                        out=wlhs[:, :SG, :Fl].rearrange("p g (h c) -> p g h c", h=H),
                        in0=xgv.rearrange("p g (h c) -> p g h c", h=H),
                        in1=wlhs[:, :SG, Fl:Fl + H][:, :, :, None].to_broadcast([P, SG, H, Cl]),
                        op=OP.mult)

                    if STAGE < 5:
                        continue
                    newly = []
                    for b in m.sb_blocks[s]:
                        bp = m.blk_pairs[b]
                        psB = ppB.tile([P, F3 + H], f32, tag="psB", space="PSUM")
                        for i, (_, p, sl) in enumerate(bp):
                            nc.tensor.matmul(psB[:, :Fl + H], lhsT=S[:, p, :],
                                             rhs=wlhs[:, sl, :Fl + H],
                                             start=(i == 0), stop=(i == len(bp) - 1))
                        hbB = hpool.tile([P, F3 + H], f32, tag="hbB")
                        nc.scalar.copy(hbB[:, :Fl + H], psB[:, :Fl + H])
                        newly.append((b, hbB))
                    for b, hbB in pending:
                        emit_tail(b, hbB)
                    pending = newly
                for b, hbB in pending:
                    emit_tail(b, hbB)

    nc.compile()
    return nc


def _prep_inputs(x, edge_index, Ws, atts):
    m = _preprocess(edge_index)
    ident = np.eye(P, dtype=np.float32).astype(BF16)
    common = {"ident": ident}
    for li, ((Wl, Wr), a) in enumerate(zip(Ws, atts)):
        Fl = Wl.shape[1]
        common[f"W{li + 1}"] = np.concatenate([Wl, Wr], axis=1).astype(BF16)
        a_flat = np.asarray(a).reshape(Fl).astype(np.float32)
        common[f"arep{li + 1}"] = np.broadcast_to(a_flat, (P, Fl)).astype(BF16).copy()
    in_maps = []
    for c in range(NCORES):
        d = dict(common)
        d["xT"] = x[c * SHARD:(c + 1) * SHARD].T.astype(BF16).copy()
        d["srcw"] = m.src_w[c]
        d["stw"] = m.stw[c]
        d["sw"] = m.sw[c]
        in_maps.append(d)
    return in_maps, m


def kernel(x, edge_index, W1l, W1r, a1, b1, W2l, W2r, a2, b2, W3l, W3r, a3, b3,
           _trace=False):
    from concourse.bass_utils import run_bass_kernel_spmd

    x = np.asarray(x, dtype=np.float32)
    edge_index = np.asarray(edge_index, dtype=np.int32)
    in_maps, m = _prep_inputs(
        x, edge_index,
        [(np.asarray(W1l), np.asarray(W1r)), (np.asarray(W2l), np.asarray(W2r)),
         (np.asarray(W3l), np.asarray(W3r))],
        [a1, a2, a3])
    for c in range(NCORES):
        in_maps[c]["brep1"] = np.broadcast_to(np.asarray(b1, np.float32), (P, F1)).astype(BF16).copy()
        in_maps[c]["brep2"] = np.broadcast_to(np.asarray(b2, np.float32), (P, F1)).astype(BF16).copy()
        in_maps[c]["brep3"] = np.broadcast_to(np.asarray(b3, np.float32), (P, C3)).astype(np.float32).copy()

    key = (m.Gtot, tuple(m.sb_g), tuple(tuple(o) for o in m.sb_cls_off))
    if key not in _cache:
        _cache.clear()
        _cache[key] = _build_program(m)
    nc = _cache[key]

    res = run_bass_kernel_spmd(nc, in_maps, core_ids=list(range(NCORES)),
                               trace=_trace)
    out = np.concatenate([res.results[c]["out_shard"] for c in range(NCORES)], axis=0)
    kernel._last_result = res
    return out

